# revision 1
# baseline (speedup 1.0000x reference)
import numpy as np
import ml_dtypes

B, S, I, H, C = 64, 512, 256, 512, 10
NCORES = 8
BL = B // NCORES
CH = 32
D = 14
NCH = S // CH

_cache = {}


def _build_nc():
    from collections import deque

    import concourse.bass as bass
    import concourse.bacc as bacc
    import concourse.tile as tile
    from concourse.bass import mybir

    f32 = mybir.dt.float32
    bf16 = mybir.dt.bfloat16
    Tanh = mybir.ActivationFunctionType.Tanh

    nc = bacc.Bacc("TRN2", target_bir_lowering=False, debug=False, num_devices=NCORES)

    xT_d = nc.dram_tensor("xT", [128, 2, S * BL], bf16, kind="ExternalInput")
    wih0_d = nc.dram_tensor("wih0", [128, 2, 4, 128], bf16, kind="ExternalInput")
    whh0_d = nc.dram_tensor("whh0", [128, 4, 4, 128], bf16, kind="ExternalInput")
    wih1_d = nc.dram_tensor("wih1", [128, 4, 4, 128], bf16, kind="ExternalInput")
    whh1_d = nc.dram_tensor("whh1", [128, 4, 4, 128], bf16, kind="ExternalInput")
    wfc_d = nc.dram_tensor("wfc", [128, 4, C], bf16, kind="ExternalInput")
    b0_d = nc.dram_tensor("b0", [128, 4], f32, kind="ExternalInput")
    b1_d = nc.dram_tensor("b1", [128, 4], f32, kind="ExternalInput")
    bfc_d = nc.dram_tensor("bfc", [C, 1], f32, kind="ExternalInput")
    id_d = nc.dram_tensor("ident", [128, 128], bf16, kind="ExternalInput")
    out_d = nc.dram_tensor("out", [C, BL], f32, kind="ExternalOutput")

    with tile.TileContext(nc) as tc:
        with tc.tile_pool(name="sb", bufs=1) as sb, tc.tile_pool(
            name="ps", bufs=1, space="PSUM"
        ) as psp:
            xT = sb.tile([128, 2, S * BL], bf16)
            pre0 = sb.tile([128, S, 4, BL], bf16)
            out0 = sb.tile([128, S, 4, BL], bf16)
            pre1 = sb.tile([128, S, 4, BL], bf16)
            wih0 = sb.tile([128, 2, 4, 128], bf16)
            whh0 = sb.tile([128, 4, 4, 128], bf16)
            wih1 = sb.tile([128, 4, 4, 128], bf16)
            whh1 = sb.tile([128, 4, 4, 128], bf16)
            wfc = sb.tile([128, 4, C], bf16)
            b0 = sb.tile([128, 4], f32)
            b1 = sb.tile([128, 4], f32)
            bfc = sb.tile([C, 1], f32)
            ident = sb.tile([128, 128], bf16)
            h1 = sb.tile([128, 2, 4, BL], bf16)
            fco = sb.tile([C, BL], f32)

            nc.sync.dma_start(wih0[:], wih0_d[:])
            nc.sync.dma_start(
                xT[:, :, 0 : CH * BL], xT_d[:, :, 0 : CH * BL]
            )
            nc.sync.dma_start(b0[:], b0_d[:])
            nc.sync.dma_start(ident[:], id_d[:])
            nc.sync.dma_start(
                xT[:, :, CH * BL : 4 * CH * BL], xT_d[:, :, CH * BL : 4 * CH * BL]
            )
            nc.sync.dma_start(whh0[:], whh0_d[:])
            nc.sync.dma_start(
                xT[:, :, 4 * CH * BL :], xT_d[:, :, 4 * CH * BL :]
            )
            for t_sb, t_d in [
                (wih1, wih1_d), (whh1, whh1_d), (b1, b1_d),
                (wfc, wfc_d), (bfc, bfc_d),
            ]:
                nc.sync.dma_start(t_sb[:], t_d[:])

            gps = [psp.tile([128, 64, BL], f32, name=f"gps{i}") for i in range(4)]
            sps = [psp.tile([128, 4, 4, BL], f32, name=f"sps{i}") for i in range(4)]

            def g0_group(k, jc):
                t0 = k * CH
                ps = gps[jc]
                for kc in range(2):
                    nc.tensor.matmul(
                        ps[:, 0:CH, :],
                        wih0[:, kc, jc, :],
                        xT[:, kc, t0 * BL : (t0 + CH) * BL],
                        start=(kc == 0),
                        stop=(kc == 1),
                    )
                nc.vector.tensor_scalar_add(
                    pre0[:, t0 : t0 + CH, jc, :], ps[:, 0:CH, :], b0[:, jc : jc + 1]
                )

            # pre1 in 8-step groups (N=64 is still issue-bound, so this
            # halves the pre1 instruction count) so layer 1 can trail layer 0
            # by only D=16 steps: every iteration of lag costs a full ~1.2us
            # latency cycle, so shrinking D removes (36-16)=20 iterations.
            def g1_group(k8, jc):
                t0 = k8 * 8
                ps = gps[jc]
                for kc in range(4):
                    nc.tensor.matmul(
                        ps[:, 0:8, :],
                        wih1[:, kc, jc, :],
                        out0[:, t0 : t0 + 8, kc, :],
                        start=(kc == 0),
                        stop=(kc == 3),
                    )
                nc.vector.tensor_scalar_add(
                    pre1[:, t0 : t0 + 8, jc, :], ps[:, 0:8, :], b1[:, jc : jc + 1]
                )

            # (chunk, jc, emit_fn); q1 items gated by min_t
            q0 = deque(
                (k, jc, g0_group) for k in range(1, NCH) for jc in range(4)
            )
            q1 = deque(
                (8 * k8 + 8, k8, jc, g1_group)
                for k8 in range(S // 8) for jc in range(4)
            )

            def pop(t, n=1):
                for _ in range(n):
                    if q1 and q1[0][0] <= t:
                        _, k, jc, fn = q1.popleft()
                        fn(k, jc)
                    elif q0:
                        k, jc, fn = q0.popleft()
                        fn(k, jc)

            def drain_q0(k):
                while q0 and q0[0][0] <= k:
                    kk, jc, fn = q0.popleft()
                    fn(kk, jc)

            def drain_q1(k4):
                while q1 and q1[0][1] <= k4:
                    _, kk, jc, fn = q1.popleft()
                    fn(kk, jc)

            def scan_step(t, pre, whh, ps, h_out, h_in_fn):
                sl = t % 4
                if sl == 0:
                    # inject pre for this step AND the next 3 (same PSUM bank)
                    nc.tensor.matmul(
                        ps[:, 0:4, :, :], ident[:], pre[:, t : t + 4, :, :],
                        start=True, stop=False,
                    )
                for kc in range(4):
                    for jc in range(4):
                        nc.tensor.matmul(
                            ps[:, sl, jc, :],
                            whh[:, kc, jc, :],
                            h_in_fn(kc),
                            start=False,
                            stop=(kc == 3),
                        )
                nc.scalar.activation(h_out, ps[:, sl, :, :], Tanh)

            def l0_step(t):
                ps = sps[(t // 4) % 2]
                if t == 0:
                    nc.tensor.matmul(
                        ps[:, 0:4, :, :], ident[:], pre0[:, 0:4, :, :],
                        start=True, stop=False,
                    )
                    nc.scalar.activation(out0[:, 0, :, :], ps[:, 0, :, :], Tanh)
                else:
                    scan_step(
                        t, pre0, whh0, ps,
                        out0[:, t, :, :],
                        lambda kc: out0[:, t - 1, kc, :],
                    )

            def l1_step(t):
                ps = sps[2 + (t // 4) % 2]
                if t == 0:
                    nc.tensor.matmul(
                        ps[:, 0:4, :, :], ident[:], pre1[:, 0:4, :, :],
                        start=True, stop=False,
                    )
                    nc.scalar.activation(h1[:, 0, :, :], ps[:, 0, :, :], Tanh)
                else:
                    scan_step(
                        t, pre1, whh1, ps,
                        h1[:, t % 2, :, :],
                        lambda kc: h1[:, (t - 1) % 2, kc, :],
                    )

            for jc in range(4):
                g0_group(0, jc)

            for t in range(S + D):
                if t < S:
                    drain_q0(t // CH)
                    l0_step(t)
                pop(t, 1)
                if t >= D:
                    s = t - D
                    drain_q1(s // 8)
                    l1_step(s)
                pop(t, 1)

            fps = gps[0]
            for kc in range(4):
                nc.tensor.matmul(
                    fps[0:C, 0, :], wfc[:, kc, :], h1[:, 1, kc, :],
                    start=(kc == 0), stop=(kc == 3),
                )
            nc.vector.tensor_scalar_add(fco[:], fps[0:C, 0, :], bfc[:])
            nc.sync.dma_start(out_d[:], fco[:])

    nc.compile()
    return nc


def _prep_inputs(inputs):
    bf = ml_dtypes.bfloat16
    w_ih0 = inputs["w_ih0"]
    w_hh0 = inputs["w_hh0"]
    w_ih1 = inputs["w_ih1"]
    w_hh1 = inputs["w_hh1"]
    w_fc = inputs["w_fc"]

    def lhsT_4(w, n_kc):
        # w: [512, n_kc*128] -> [kp, kc, jc, jp]
        return np.ascontiguousarray(
            w.reshape(4, 128, n_kc, 128).transpose(3, 2, 0, 1)
        ).astype(bf)

    shared = {
        "wih0": lhsT_4(w_ih0, 2),
        "whh0": lhsT_4(w_hh0, 4),
        "wih1": lhsT_4(w_ih1, 4),
        "whh1": lhsT_4(w_hh1, 4),
        "wfc": np.ascontiguousarray(w_fc.reshape(C, 4, 128).transpose(2, 1, 0)).astype(bf),
        "b0": np.ascontiguousarray(
            (inputs["b_ih0"] + inputs["b_hh0"]).reshape(4, 128).T
        ).astype(np.float32),
        "b1": np.ascontiguousarray(
            (inputs["b_ih1"] + inputs["b_hh1"]).reshape(4, 128).T
        ).astype(np.float32),
        "bfc": inputs["b_fc"].reshape(C, 1).astype(np.float32),
        "ident": np.eye(128, dtype=np.float32).astype(bf),
    }
    x = inputs["x"]
    in_maps = []
    for c in range(NCORES):
        xs = x[c * BL : (c + 1) * BL]  # [b, t, i]
        xT = (
            np.ascontiguousarray(
                xs.transpose(2, 1, 0).reshape(2, 128, S * BL).transpose(1, 0, 2)
            )
        ).astype(bf)
        m = dict(shared)
        m["xT"] = xT
        in_maps.append(m)
    return in_maps


def kernel(**inputs):
    from concourse import bass_utils

    if "nc" not in _cache:
        _cache["nc"] = _build_nc()
    nc = _cache["nc"]
    in_maps = _prep_inputs(inputs)
    res = bass_utils.run_bass_kernel_spmd(nc, in_maps, core_ids=list(range(NCORES)))
    y = np.concatenate(
        [np.asarray(res.results[c]["out"]).T for c in range(NCORES)], axis=0
    )
    return y.astype(np.float32)



# revision 13
# speedup vs baseline: 1.0259x; 1.0259x over previous
import numpy as np
import ml_dtypes

B, S, I, H, C = 64, 512, 256, 512, 10
NCORES = 8
BL = B // NCORES
D = 16          # l1 lag behind l0, in steps
CH = 8          # steps per pre-GEMM chunk (= one PSUM bank)
NCHK = S // CH

_cache = {}


def _build_nc():
    from collections import deque

    import concourse.bass as bass
    import concourse.bacc as bacc
    from concourse.bass import mybir

    f32 = mybir.dt.float32
    bf16 = mybir.dt.bfloat16
    Tanh = mybir.ActivationFunctionType.Tanh

    nc = bacc.Bacc("TRN2", target_bir_lowering=False, debug=False, num_devices=NCORES)

    xT_d = nc.dram_tensor("xT", [128, 2, S * BL], bf16, kind="ExternalInput")
    wih0_d = nc.dram_tensor("wih0", [128, 2, 4, 128], bf16, kind="ExternalInput")
    whh0_d = nc.dram_tensor("whh0", [128, 4, 4, 128], bf16, kind="ExternalInput")
    wih1_d = nc.dram_tensor("wih1", [128, 4, 4, 128], bf16, kind="ExternalInput")
    whh1_d = nc.dram_tensor("whh1", [128, 4, 4, 128], bf16, kind="ExternalInput")
    wfc_d = nc.dram_tensor("wfc", [128, 4, C], bf16, kind="ExternalInput")
    b0_d = nc.dram_tensor("b0", [4, 128], bf16, kind="ExternalInput")
    b1_d = nc.dram_tensor("b1", [4, 128], bf16, kind="ExternalInput")
    ones_d = nc.dram_tensor("sel", [4, CH, 4, BL], bf16, kind="ExternalInput")
    bfc_d = nc.dram_tensor("bfc", [C, 1], f32, kind="ExternalInput")
    out_d = nc.dram_tensor("out", [C, BL], f32, kind="ExternalOutput")
    import os
    DBG = os.environ.get("KDBG") == "1"
    if DBG:
        hist_d = nc.dram_tensor("hist", [128, S, 4, BL], bf16, kind="ExternalOutput")


    from contextlib import ExitStack

    with ExitStack() as stack:
        e = stack.enter_context
        block = e(nc.Block())
        dmas = e(nc.semaphore("dmas"))
        h0 = e(nc.semaphore("h0"))
        h1 = e(nc.semaphore("h1"))
        a0 = e(nc.semaphore("a0"))
        a1 = e(nc.semaphore("a1"))
        fcs = e(nc.semaphore("fcs"))
        vs = e(nc.semaphore("vs"))
        xT = e(nc.sbuf_tensor("xT_s", [128, 2, S * BL], bf16))
        wih0 = e(nc.sbuf_tensor("wih0_s", [128, 2, 4, 128], bf16))
        whh0 = e(nc.sbuf_tensor("whh0_s", [128, 4, 4, 128], bf16))
        wih1 = e(nc.sbuf_tensor("wih1_s", [128, 4, 4, 128], bf16))
        whh1 = e(nc.sbuf_tensor("whh1_s", [128, 4, 4, 128], bf16))
        wfc = e(nc.sbuf_tensor("wfc_s", [128, 4, C], bf16))
        b0r = e(nc.sbuf_tensor("b0r", [4, 128], bf16))
        b1r = e(nc.sbuf_tensor("b1r", [4, 128], bf16))
        onesv = e(nc.sbuf_tensor("selv", [4, CH, 4, BL], bf16))
        bfcv = e(nc.sbuf_tensor("bfcv_s", [C, 1], f32))
        hist0 = e(nc.sbuf_tensor("hist0", [128, S, 4, BL], bf16))
        h1r = e(nc.sbuf_tensor("h1r", [128, 2, 4, BL], bf16))
        fco = e(nc.sbuf_tensor("fco", [C, BL], f32))
        ps0a = e(nc.psum_tensor("ps0a", [128, CH, 4, BL], f32))
        ps0b = e(nc.psum_tensor("ps0b", [128, CH, 4, BL], f32))
        ps1a = e(nc.psum_tensor("ps1a", [128, CH, 4, BL], f32))
        ps1b = e(nc.psum_tensor("ps1b", [128, CH, 4, BL], f32))
        ps0 = [ps0a, ps0b]
        ps1 = [ps1a, ps1b]
        fcps = e(nc.psum_tensor("fcps", [128, BL], f32))
        # DMA milestones (each dma_start incs dmas by 16)
        # M1 (=64):  wih0, b0r, onesv, xT[0:64 steps]
        # M2 (=80):  whh0
        # M3 (=128): wih1, b1r, whh1
        # M4 (=144): xT rest
        # M5 (=176): wfc, bfcv
        @block.sync
        def _(sync):
            sync.dma_start(out=wih0[:], in_=wih0_d[:]).then_inc(dmas, 16)
            sync.dma_start(out=b0r[:], in_=b0_d[:]).then_inc(dmas, 16)
            sync.dma_start(out=onesv[:], in_=ones_d[:]).then_inc(dmas, 16)
            sync.dma_start(
                out=xT[:, :, 0 : 64 * BL], in_=xT_d[:, :, 0 : 64 * BL]
            ).then_inc(dmas, 16)
            sync.dma_start(out=whh0[:], in_=whh0_d[:]).then_inc(dmas, 16)
            sync.dma_start(out=wih1[:], in_=wih1_d[:]).then_inc(dmas, 16)
            sync.dma_start(out=b1r[:], in_=b1_d[:]).then_inc(dmas, 16)
            sync.dma_start(out=whh1[:], in_=whh1_d[:]).then_inc(dmas, 16)
            sync.dma_start(
                out=xT[:, :, 64 * BL :], in_=xT_d[:, :, 64 * BL :]
            ).then_inc(dmas, 16)
            sync.dma_start(out=wfc[:], in_=wfc_d[:]).then_inc(dmas, 16)
            sync.dma_start(out=bfcv[:], in_=bfc_d[:]).then_inc(dmas, 16)
            sync.wait_ge(vs, 1)
            sync.dma_start(out=out_d[:], in_=fco[:]).then_inc(dmas, 16)
            if DBG:
                sync.dma_start(out=hist_d[:], in_=hist0[:]).then_inc(dmas, 16)
                sync.wait_ge(dmas, 208)
            else:
                sync.wait_ge(dmas, 192)

        @block.tensor
        def _(te):
            te.wait_ge(dmas, 64)

            def gchunk0(c):
                # l0 pre chunk c: whole-bank bias (start=True) + W_ih0 @ x
                t0 = c * CH
                bank = ps0[c % 2]
                last = te.matmul(
                    bank[:, :, :, :], b0r[:, :], onesv[:, :, :, :],
                    start=True, stop=False, skip_group_check=True,
                )
                for jc in range(4):
                    for kc in range(2):
                        last = te.matmul(
                            bank[:, :, jc, :],
                            wih0[:, kc, jc, :],
                            xT[:, kc, t0 * BL : (t0 + CH) * BL],
                            start=False, stop=False, skip_group_check=True,
                        )
                return last

            def gchunk1(c):
                t0 = c * CH
                bank = ps1[c % 2]
                last = te.matmul(
                    bank[:, :, :, :], b1r[:, :], onesv[:, :, :, :],
                    start=True, stop=False, skip_group_check=True,
                )
                for jc in range(4):
                    for kc in range(4):
                        last = te.matmul(
                            bank[:, :, jc, :],
                            wih1[:, kc, jc, :],
                            hist0[:, t0 : t0 + CH, kc, :],
                            start=False, stop=False, skip_group_check=True,
                        )
                return last

            # GEMM work queue: list of (min_slot, emit_fn) in issue order.
            def emit_g0(c):
                def f():
                    if c == 8:
                        te.wait_ge(dmas, 144)  # xT rest
                    if c >= 2:
                        te.wait_ge(a0, max(0, CH * (c - 1)))
                    last = gchunk0(c)
                    if c == 0:
                        last.then_inc(h0)
                return f

            def emit_g1(c):
                def f():
                    if c == 0:
                        te.wait_ge(dmas, 128)  # wih1, b1r, whh1
                    te.wait_ge(a0, CH * c + CH)  # h0 history through chunk
                    if c >= 2:
                        te.wait_ge(a1, max(0, CH * (c - 1)))
                    last = gchunk1(c)
                    if c == 0:
                        last.then_inc(h1)
                return f

            q = deque()
            for c in range(2, NCHK):
                q.append((CH * c - 7, emit_g0(c)))
            q1 = deque()
            for c in range(NCHK):
                q1.append((CH * c + D - 5, emit_g1(c)))

            def pop_ready(t, n):
                done = 0
                while done < n:
                    if q1 and q1[0][0] <= t:
                        q1.popleft()[1]()
                    elif q and q[0][0] <= t:
                        q.popleft()[1]()
                    else:
                        break
                    done += 1

            # prime: l0 chunks 0 and 1
            emit_g0(0)()
            te.wait_ge(dmas, 80)  # whh0
            emit_g0(1)()

            for t in range(S + D):
                # l0 scan step t
                if 0 < t < S:
                    te.wait_ge(a0, t)
                    for jc in range(4):
                        for kc in range(4):
                            mm = te.matmul(
                                ps0[(t // CH) % 2][:, t % CH, jc, :],
                                whh0[:, kc, jc, :],
                                hist0[:, t - 1, kc, :],
                                start=False, stop=(kc == 3),
                                skip_group_check=True,
                            )
                    mm.then_inc(h0)
                pop_ready(t, 1)
                # l1 scan step s
                s = t - D
                if 0 < s < S:
                    te.wait_ge(a1, s)
                    for jc in range(4):
                        for kc in range(4):
                            mm = te.matmul(
                                ps1[(s // CH) % 2][:, s % CH, jc, :],
                                whh1[:, kc, jc, :],
                                h1r[:, (s - 1) % 2, kc, :],
                                start=False, stop=(kc == 3),
                                skip_group_check=True,
                            )
                    mm.then_inc(h1)
                pop_ready(t, 1)

            # FC tail
            te.wait_ge(a1, S)
            te.wait_ge(dmas, 176)
            for kc in range(4):
                mm = te.matmul(
                    fcps[0:C, :], wfc[:, kc, :], h1r[:, (S - 1) % 2, kc, :],
                    start=(kc == 0), stop=(kc == 3),
                )
            mm.then_inc(fcs)

        @block.scalar
        def _(sc):
            for t in range(S + D):
                if t < S:
                    sc.wait_ge(h0, t + 1)
                    sc.activation(
                        hist0[:, t, :, :],
                        ps0[(t // CH) % 2][:, t % CH, :, :],
                        Tanh,
                    ).then_inc(a0)
                s = t - D
                if 0 <= s < S:
                    sc.wait_ge(h1, s + 1)
                    sc.activation(
                        h1r[:, s % 2, :, :],
                        ps1[(s // CH) % 2][:, s % CH, :, :],
                        Tanh,
                    ).then_inc(a1)

        @block.vector
        def _(ve):
            ve.wait_ge(fcs, 1)
            ve.tensor_scalar_add(fco[:, :], fcps[0:C, :], bfcv[:, 0:1]).then_inc(vs)

    nc.compile()
    return nc


def _make_sel():
    bf = ml_dtypes.bfloat16
    sel = np.zeros((4, CH, 4, BL), dtype=np.float32)
    for jc in range(4):
        sel[jc, :, jc, :] = 1.0
    return sel.astype(bf)


def _prep_inputs(inputs):
    bf = ml_dtypes.bfloat16
    w_ih0 = inputs["w_ih0"]
    w_hh0 = inputs["w_hh0"]
    w_ih1 = inputs["w_ih1"]
    w_hh1 = inputs["w_hh1"]
    w_fc = inputs["w_fc"]

    def lhsT_4(w, n_kc):
        # w: [512, n_kc*128] -> [kp, kc, jc, jp]
        return np.ascontiguousarray(
            w.reshape(4, 128, n_kc, 128).transpose(3, 2, 0, 1)
        ).astype(bf)

    shared = {
        "wih0": lhsT_4(w_ih0, 2),
        "whh0": lhsT_4(w_hh0, 4),
        "wih1": lhsT_4(w_ih1, 4),
        "whh1": lhsT_4(w_hh1, 4),
        "wfc": np.ascontiguousarray(
            w_fc.reshape(C, 4, 128).transpose(2, 1, 0)
        ).astype(bf),
        "b0": np.ascontiguousarray(
            (inputs["b_ih0"] + inputs["b_hh0"]).reshape(4, 128)
        ).astype(bf),
        "b1": np.ascontiguousarray(
            (inputs["b_ih1"] + inputs["b_hh1"]).reshape(4, 128)
        ).astype(bf),
        "sel": _make_sel(),
        "bfc": inputs["b_fc"].reshape(C, 1).astype(np.float32),
    }
    x = inputs["x"]
    in_maps = []
    for c in range(NCORES):
        xs = x[c * BL : (c + 1) * BL]  # [b, t, i]
        xT = (
            np.ascontiguousarray(
                xs.transpose(2, 1, 0).reshape(2, 128, S * BL).transpose(1, 0, 2)
            )
        ).astype(bf)
        m = dict(shared)
        m["xT"] = xT
        in_maps.append(m)
    return in_maps


def kernel(**inputs):
    from concourse import bass_utils

    if "nc" not in _cache:
        _cache["nc"] = _build_nc()
    nc = _cache["nc"]
    in_maps = _prep_inputs(inputs)
    res = bass_utils.run_bass_kernel_spmd(nc, in_maps, core_ids=list(range(NCORES)))
    y = np.concatenate(
        [np.asarray(res.results[c]["out"]).T for c in range(NCORES)], axis=0
    )
    return y.astype(np.float32)


# revision 15
# speedup vs baseline: 1.0568x; 1.0301x over previous
import numpy as np
import ml_dtypes

B, S, I, H, C = 64, 512, 256, 512, 10
NCORES = 8
BL = B // NCORES
D = 16          # l1 lag behind l0, in steps
CH = 8          # steps per pre-GEMM chunk (= one PSUM bank)
NCHK = S // CH

_cache = {}


def _build_nc():
    from collections import deque

    import concourse.bass as bass
    import concourse.bacc as bacc
    from concourse.bass import mybir

    f32 = mybir.dt.float32
    bf16 = mybir.dt.bfloat16
    Tanh = mybir.ActivationFunctionType.Tanh

    nc = bacc.Bacc("TRN2", target_bir_lowering=False, debug=False, num_devices=NCORES)

    xT_d = nc.dram_tensor("xT", [128, 2, S * BL], bf16, kind="ExternalInput")
    wih0_d = nc.dram_tensor("wih0", [128, 2, 4, 128], bf16, kind="ExternalInput")
    whh0_d = nc.dram_tensor("whh0", [128, 4, 4, 128], bf16, kind="ExternalInput")
    wih1_d = nc.dram_tensor("wih1", [128, 4, 4, 128], bf16, kind="ExternalInput")
    whh1_d = nc.dram_tensor("whh1", [128, 4, 4, 128], bf16, kind="ExternalInput")
    wfc_d = nc.dram_tensor("wfc", [128, 4, C], bf16, kind="ExternalInput")
    b0_d = nc.dram_tensor("b0", [4, 128], bf16, kind="ExternalInput")
    b1_d = nc.dram_tensor("b1", [4, 128], bf16, kind="ExternalInput")
    ones_d = nc.dram_tensor("sel", [4, 4, CH, BL], bf16, kind="ExternalInput")
    bfc_d = nc.dram_tensor("bfc", [C, 1], f32, kind="ExternalInput")
    out_d = nc.dram_tensor("out", [C, BL], f32, kind="ExternalOutput")
    import os
    DBG = os.environ.get("KDBG") == "1"
    if DBG:
        hist_d = nc.dram_tensor("hist", [128, 4, S, BL], bf16, kind="ExternalOutput")


    from contextlib import ExitStack

    with ExitStack() as stack:
        e = stack.enter_context
        block = e(nc.Block())
        dmas = e(nc.semaphore("dmas"))
        h0 = e(nc.semaphore("h0"))
        h1 = e(nc.semaphore("h1"))
        a0 = e(nc.semaphore("a0"))
        a1 = e(nc.semaphore("a1"))
        fcs = e(nc.semaphore("fcs"))
        vs = e(nc.semaphore("vs"))
        xT = e(nc.sbuf_tensor("xT_s", [128, 2, S * BL], bf16))
        wih0 = e(nc.sbuf_tensor("wih0_s", [128, 2, 4, 128], bf16))
        whh0 = e(nc.sbuf_tensor("whh0_s", [128, 4, 4, 128], bf16))
        wih1 = e(nc.sbuf_tensor("wih1_s", [128, 4, 4, 128], bf16))
        whh1 = e(nc.sbuf_tensor("whh1_s", [128, 4, 4, 128], bf16))
        wfc = e(nc.sbuf_tensor("wfc_s", [128, 4, C], bf16))
        b0r = e(nc.sbuf_tensor("b0r", [4, 128], bf16))
        b1r = e(nc.sbuf_tensor("b1r", [4, 128], bf16))
        onesv = e(nc.sbuf_tensor("selv", [4, 4, CH, BL], bf16))
        bfcv = e(nc.sbuf_tensor("bfcv_s", [C, 1], f32))
        hist0 = e(nc.sbuf_tensor("hist0", [128, 4, S, BL], bf16))
        h1r = e(nc.sbuf_tensor("h1r", [128, 4, 2, BL], bf16))
        fco = e(nc.sbuf_tensor("fco", [C, BL], f32))
        ps0a = e(nc.psum_tensor("ps0a", [128, 4, CH, BL], f32))
        ps0b = e(nc.psum_tensor("ps0b", [128, 4, CH, BL], f32))
        ps1a = e(nc.psum_tensor("ps1a", [128, 4, CH, BL], f32))
        ps1b = e(nc.psum_tensor("ps1b", [128, 4, CH, BL], f32))
        ps0 = [ps0a, ps0b]
        ps1 = [ps1a, ps1b]
        fcps = e(nc.psum_tensor("fcps", [128, BL], f32))
        # DMA milestones (each dma_start incs dmas by 16)
        # M1 (=64):  wih0, b0r, onesv, xT[0:64 steps]
        # M2 (=80):  whh0
        # M3 (=128): wih1, b1r, whh1
        # M4 (=144): xT rest
        # M5 (=176): wfc, bfcv
        @block.sync
        def _(sync):
            sync.dma_start(out=wih0[:], in_=wih0_d[:]).then_inc(dmas, 16)
            sync.dma_start(out=b0r[:], in_=b0_d[:]).then_inc(dmas, 16)
            sync.dma_start(out=onesv[:], in_=ones_d[:]).then_inc(dmas, 16)
            sync.dma_start(
                out=xT[:, :, 0 : 64 * BL], in_=xT_d[:, :, 0 : 64 * BL]
            ).then_inc(dmas, 16)
            sync.dma_start(out=whh0[:], in_=whh0_d[:]).then_inc(dmas, 16)
            sync.dma_start(out=wih1[:], in_=wih1_d[:]).then_inc(dmas, 16)
            sync.dma_start(out=b1r[:], in_=b1_d[:]).then_inc(dmas, 16)
            sync.dma_start(out=whh1[:], in_=whh1_d[:]).then_inc(dmas, 16)
            sync.dma_start(
                out=xT[:, :, 64 * BL :], in_=xT_d[:, :, 64 * BL :]
            ).then_inc(dmas, 16)
            sync.dma_start(out=wfc[:], in_=wfc_d[:]).then_inc(dmas, 16)
            sync.dma_start(out=bfcv[:], in_=bfc_d[:]).then_inc(dmas, 16)
            sync.wait_ge(vs, 1)
            sync.dma_start(out=out_d[:], in_=fco[:]).then_inc(dmas, 16)
            if DBG:
                sync.dma_start(out=hist_d[:], in_=hist0[:]).then_inc(dmas, 16)
                sync.wait_ge(dmas, 208)
            else:
                sync.wait_ge(dmas, 192)

        @block.tensor
        def _(te):
            te.wait_ge(dmas, 64)

            def sel0(c):
                def f():
                    if c == 8:
                        te.wait_ge(dmas, 144)  # xT rest
                    if c >= 2:
                        te.wait_ge(a0, max(0, CH * (c - 1)))
                    te.matmul(
                        ps0[c % 2][:, :, :, :], b0r[:, :], onesv[:, :, :, :],
                        start=True, stop=False, skip_group_check=True,
                    )
                return f

            def g0(c, jc):
                def f():
                    last = None
                    for kc in range(2):
                        last = te.matmul(
                            ps0[c % 2][:, jc, :, :],
                            wih0[:, kc, jc, :],
                            xT[:, kc, c * CH * BL : (c + 1) * CH * BL],
                            start=False, stop=False, skip_group_check=True,
                        )
                    if c == 0 and jc == 3:
                        last.then_inc(h0)
                return f

            def sel1(c):
                def f():
                    if c == 0:
                        te.wait_ge(dmas, 128)  # wih1, b1r, whh1
                    if c >= 2:
                        te.wait_ge(a1, max(0, CH * (c - 1)))
                    te.matmul(
                        ps1[c % 2][:, :, :, :], b1r[:, :], onesv[:, :, :, :],
                        start=True, stop=False, skip_group_check=True,
                    )
                return f

            def g1(c, jc):
                def f():
                    if jc == 0:
                        te.wait_ge(a0, CH * c + CH)  # h0 history through chunk
                    last = None
                    for kc in range(4):
                        last = te.matmul(
                            ps1[c % 2][:, jc, :, :],
                            wih1[:, kc, jc, :],
                            hist0[:, kc, c * CH : (c + 1) * CH, :],
                            start=False, stop=False, skip_group_check=True,
                        )
                    if c == 0 and jc == 3:
                        last.then_inc(h1)
                return f

            q = deque()
            for c in range(2, NCHK):
                q.append((CH * c - 7, sel0(c)))
                for jc in range(4):
                    q.append((CH * c - 7, g0(c, jc)))
            q1 = deque()
            for c in range(NCHK):
                q1.append((CH * c + D - 7, sel1(c)))
                for jc in range(4):
                    q1.append((CH * c + D - 6, g1(c, jc)))

            def pop_ready(t, n):
                done = 0
                while done < n:
                    if q1 and q1[0][0] <= t:
                        q1.popleft()[1]()
                    elif q and q[0][0] <= t:
                        q.popleft()[1]()
                    else:
                        break
                    done += 1

            # prime: l0 chunks 0 and 1
            sel0(0)()
            for jc in range(4):
                g0(0, jc)()
            te.wait_ge(dmas, 80)  # whh0
            sel0(1)()
            for jc in range(4):
                g0(1, jc)()

            for t in range(S + D):
                # l0 scan step t
                if 0 < t < S:
                    te.wait_ge(a0, t)
                    for jc in range(4):
                        for kc in range(4):
                            mm = te.matmul(
                                ps0[(t // CH) % 2][:, jc, t % CH, :],
                                whh0[:, kc, jc, :],
                                hist0[:, kc, t - 1, :],
                                start=False, stop=(kc == 3),
                                skip_group_check=True,
                            )
                    mm.then_inc(h0)
                pop_ready(t, 2)
                # l1 scan step s
                s = t - D
                if 0 < s < S:
                    te.wait_ge(a1, s)
                    for jc in range(4):
                        for kc in range(4):
                            mm = te.matmul(
                                ps1[(s // CH) % 2][:, jc, s % CH, :],
                                whh1[:, kc, jc, :],
                                h1r[:, kc, (s - 1) % 2, :],
                                start=False, stop=(kc == 3),
                                skip_group_check=True,
                            )
                    mm.then_inc(h1)
                pop_ready(t, 2)

            # FC tail
            te.wait_ge(a1, S)
            te.wait_ge(dmas, 176)
            for kc in range(4):
                mm = te.matmul(
                    fcps[0:C, :], wfc[:, kc, :], h1r[:, kc, (S - 1) % 2, :],
                    start=(kc == 0), stop=(kc == 3),
                )
            mm.then_inc(fcs)

        @block.scalar
        def _(sc):
            for t in range(S + D):
                if t < S:
                    sc.wait_ge(h0, t + 1)
                    sc.activation(
                        hist0[:, :, t, :],
                        ps0[(t // CH) % 2][:, :, t % CH, :],
                        Tanh,
                    ).then_inc(a0)
                s = t - D
                if 0 <= s < S:
                    sc.wait_ge(h1, s + 1)
                    sc.activation(
                        h1r[:, :, s % 2, :],
                        ps1[(s // CH) % 2][:, :, s % CH, :],
                        Tanh,
                    ).then_inc(a1)

        @block.vector
        def _(ve):
            ve.wait_ge(fcs, 1)
            ve.tensor_scalar_add(fco[:, :], fcps[0:C, :], bfcv[:, 0:1]).then_inc(vs)

    nc.compile()
    return nc


def _make_sel():
    bf = ml_dtypes.bfloat16
    sel = np.zeros((4, 4, CH, BL), dtype=np.float32)
    for jc in range(4):
        sel[jc, jc, :, :] = 1.0
    return sel.astype(bf)


def _prep_inputs(inputs):
    bf = ml_dtypes.bfloat16
    w_ih0 = inputs["w_ih0"]
    w_hh0 = inputs["w_hh0"]
    w_ih1 = inputs["w_ih1"]
    w_hh1 = inputs["w_hh1"]
    w_fc = inputs["w_fc"]

    def lhsT_4(w, n_kc):
        # w: [512, n_kc*128] -> [kp, kc, jc, jp]
        return np.ascontiguousarray(
            w.reshape(4, 128, n_kc, 128).transpose(3, 2, 0, 1)
        ).astype(bf)

    shared = {
        "wih0": lhsT_4(w_ih0, 2),
        "whh0": lhsT_4(w_hh0, 4),
        "wih1": lhsT_4(w_ih1, 4),
        "whh1": lhsT_4(w_hh1, 4),
        "wfc": np.ascontiguousarray(
            w_fc.reshape(C, 4, 128).transpose(2, 1, 0)
        ).astype(bf),
        "b0": np.ascontiguousarray(
            (inputs["b_ih0"] + inputs["b_hh0"]).reshape(4, 128)
        ).astype(bf),
        "b1": np.ascontiguousarray(
            (inputs["b_ih1"] + inputs["b_hh1"]).reshape(4, 128)
        ).astype(bf),
        "sel": _make_sel(),
        "bfc": inputs["b_fc"].reshape(C, 1).astype(np.float32),
    }
    x = inputs["x"]
    in_maps = []
    for c in range(NCORES):
        xs = x[c * BL : (c + 1) * BL]  # [b, t, i]
        xT = (
            np.ascontiguousarray(
                xs.transpose(2, 1, 0).reshape(2, 128, S * BL).transpose(1, 0, 2)
            )
        ).astype(bf)
        m = dict(shared)
        m["xT"] = xT
        in_maps.append(m)
    return in_maps


def kernel(**inputs):
    from concourse import bass_utils

    if "nc" not in _cache:
        _cache["nc"] = _build_nc()
    nc = _cache["nc"]
    in_maps = _prep_inputs(inputs)
    res = bass_utils.run_bass_kernel_spmd(nc, in_maps, core_ids=list(range(NCORES)))
    y = np.concatenate(
        [np.asarray(res.results[c]["out"]).T for c in range(NCORES)], axis=0
    )
    return y.astype(np.float32)


# revision 16
# speedup vs baseline: 1.0825x; 1.0243x over previous
import numpy as np
import ml_dtypes

B, S, I, H, C = 64, 512, 256, 512, 10
NCORES = 8
BL = B // NCORES
D = 16          # l1 lag behind l0, in steps
CH = 8          # steps per pre-GEMM chunk (= one PSUM bank)
NCHK = S // CH

_cache = {}


def _build_nc():
    from collections import deque

    import concourse.bass as bass
    import concourse.bacc as bacc
    from concourse.bass import mybir

    f32 = mybir.dt.float32
    bf16 = mybir.dt.bfloat16
    Tanh = mybir.ActivationFunctionType.Tanh

    nc = bacc.Bacc("TRN2", target_bir_lowering=False, debug=False, num_devices=NCORES)

    xT_d = nc.dram_tensor("xT", [128, 2, S * BL], bf16, kind="ExternalInput")
    wih0_d = nc.dram_tensor("wih0", [128, 2, 4, 128], bf16, kind="ExternalInput")
    whh0_d = nc.dram_tensor("whh0", [128, 4, 4, 128], bf16, kind="ExternalInput")
    wih1_d = nc.dram_tensor("wih1", [128, 4, 4, 128], bf16, kind="ExternalInput")
    whh1_d = nc.dram_tensor("whh1", [128, 4, 4, 128], bf16, kind="ExternalInput")
    wfc_d = nc.dram_tensor("wfc", [128, 4, C], bf16, kind="ExternalInput")
    b0_d = nc.dram_tensor("b0", [4, 128], bf16, kind="ExternalInput")
    b1_d = nc.dram_tensor("b1", [4, 128], bf16, kind="ExternalInput")
    ones_d = nc.dram_tensor("sel", [4, 4, CH, BL], bf16, kind="ExternalInput")
    bfc_d = nc.dram_tensor("bfc", [C, 1], f32, kind="ExternalInput")
    out_d = nc.dram_tensor("out", [C, BL], f32, kind="ExternalOutput")
    import os
    DBG = os.environ.get("KDBG") == "1"
    if DBG:
        hist_d = nc.dram_tensor("hist", [128, 4, S, BL], bf16, kind="ExternalOutput")


    from contextlib import ExitStack

    with ExitStack() as stack:
        e = stack.enter_context
        block = e(nc.Block())
        dmas = e(nc.semaphore("dmas"))
        h0 = e(nc.semaphore("h0"))
        h1 = e(nc.semaphore("h1"))
        a0 = e(nc.semaphore("a0"))
        a1 = e(nc.semaphore("a1"))
        fcs = e(nc.semaphore("fcs"))
        vs = e(nc.semaphore("vs"))
        xT = e(nc.sbuf_tensor("xT_s", [128, 2, S * BL], bf16))
        wih0 = e(nc.sbuf_tensor("wih0_s", [128, 2, 4, 128], bf16))
        whh0 = e(nc.sbuf_tensor("whh0_s", [128, 4, 4, 128], bf16))
        wih1 = e(nc.sbuf_tensor("wih1_s", [128, 4, 4, 128], bf16))
        whh1 = e(nc.sbuf_tensor("whh1_s", [128, 4, 4, 128], bf16))
        wfc = e(nc.sbuf_tensor("wfc_s", [128, 4, C], bf16))
        b0r = e(nc.sbuf_tensor("b0r", [4, 128], bf16))
        b1r = e(nc.sbuf_tensor("b1r", [4, 128], bf16))
        onesv = e(nc.sbuf_tensor("selv", [4, 4, CH, BL], bf16))
        bfcv = e(nc.sbuf_tensor("bfcv_s", [C, 1], f32))
        hist0 = e(nc.sbuf_tensor("hist0", [128, 4, S, BL], bf16))
        h1r = e(nc.sbuf_tensor("h1r", [128, 4, 2, BL], bf16))
        fco = e(nc.sbuf_tensor("fco", [C, BL], f32))
        ps0a = e(nc.psum_tensor("ps0a", [128, 4, CH, BL], f32))
        ps0b = e(nc.psum_tensor("ps0b", [128, 4, CH, BL], f32))
        ps1a = e(nc.psum_tensor("ps1a", [128, 4, CH, BL], f32))
        ps1b = e(nc.psum_tensor("ps1b", [128, 4, CH, BL], f32))
        ps0 = [ps0a, ps0b]
        ps1 = [ps1a, ps1b]
        fcps = e(nc.psum_tensor("fcps", [128, BL], f32))
        # DMA milestones (each dma_start incs dmas by 16)
        # M1 (=64):  wih0, b0r, onesv, xT[0:64 steps]
        # M2 (=80):  whh0
        # M3 (=128): wih1, b1r, whh1
        # M4 (=144): xT rest
        # M5 (=176): wfc, bfcv
        @block.sync
        def _(sync):
            sync.dma_start(out=wih0[:], in_=wih0_d[:]).then_inc(dmas, 16)
            sync.dma_start(out=b0r[:], in_=b0_d[:]).then_inc(dmas, 16)
            sync.dma_start(out=onesv[:], in_=ones_d[:]).then_inc(dmas, 16)
            sync.dma_start(
                out=xT[:, :, 0 : 64 * BL], in_=xT_d[:, :, 0 : 64 * BL]
            ).then_inc(dmas, 16)
            sync.dma_start(out=whh0[:], in_=whh0_d[:]).then_inc(dmas, 16)
            sync.dma_start(out=wih1[:], in_=wih1_d[:]).then_inc(dmas, 16)
            sync.dma_start(out=b1r[:], in_=b1_d[:]).then_inc(dmas, 16)
            sync.dma_start(out=whh1[:], in_=whh1_d[:]).then_inc(dmas, 16)
            sync.dma_start(
                out=xT[:, :, 64 * BL :], in_=xT_d[:, :, 64 * BL :]
            ).then_inc(dmas, 16)
            sync.dma_start(out=wfc[:], in_=wfc_d[:]).then_inc(dmas, 16)
            sync.dma_start(out=bfcv[:], in_=bfc_d[:]).then_inc(dmas, 16)
            sync.wait_ge(vs, 1)
            sync.dma_start(out=out_d[:], in_=fco[:]).then_inc(dmas, 16)
            if DBG:
                sync.dma_start(out=hist_d[:], in_=hist0[:]).then_inc(dmas, 16)
                sync.wait_ge(dmas, 208)
            else:
                sync.wait_ge(dmas, 192)

        @block.tensor
        def _(te):
            te.wait_ge(dmas, 64)

            def sel0(c):
                def f():
                    if c == 8:
                        te.wait_ge(dmas, 144)  # xT rest
                    if c >= 2:
                        te.wait_ge(a0, max(0, CH * (c - 1)))
                    te.matmul(
                        ps0[c % 2][:, :, :, :], b0r[:, :], onesv[:, :, :, :],
                        start=True, stop=False, skip_group_check=True,
                    )
                return f

            def g0(c, kc):
                def f():
                    last = None
                    for jc in range(4):
                        last = te.matmul(
                            ps0[c % 2][:, jc, :, :],
                            wih0[:, kc, jc, :],
                            xT[:, kc, c * CH * BL : (c + 1) * CH * BL],
                            start=False, stop=False, skip_group_check=True,
                        )
                    if c == 0 and kc == 1:
                        last.then_inc(h0)
                return f

            def sel1(c):
                def f():
                    if c == 0:
                        te.wait_ge(dmas, 128)  # wih1, b1r, whh1
                    if c >= 2:
                        te.wait_ge(a1, max(0, CH * (c - 1)))
                    te.matmul(
                        ps1[c % 2][:, :, :, :], b1r[:, :], onesv[:, :, :, :],
                        start=True, stop=False, skip_group_check=True,
                    )
                return f

            def g1(c, kc):
                def f():
                    if kc == 0:
                        te.wait_ge(a0, CH * c + CH)  # h0 history through chunk
                    last = None
                    for jc in range(4):
                        last = te.matmul(
                            ps1[c % 2][:, jc, :, :],
                            wih1[:, kc, jc, :],
                            hist0[:, kc, c * CH : (c + 1) * CH, :],
                            start=False, stop=False, skip_group_check=True,
                        )
                    if c == 0 and kc == 3:
                        last.then_inc(h1)
                return f

            q = deque()
            for c in range(2, NCHK):
                q.append((CH * c - 8, sel0(c)))
                for kc in range(2):
                    q.append((CH * c - 6 + kc, g0(c, kc)))
            q1 = deque()
            for c in range(NCHK):
                q1.append((CH * c + D - 8, sel1(c)))
                for kc in range(4):
                    q1.append((CH * c + D - 6 + kc, g1(c, kc)))

            def pop_ready(t, n):
                done = 0
                while done < n:
                    if q1 and q1[0][0] <= t:
                        q1.popleft()[1]()
                    elif q and q[0][0] <= t:
                        q.popleft()[1]()
                    else:
                        break
                    done += 1

            # prime: l0 chunks 0 and 1
            sel0(0)()
            for kc in range(2):
                g0(0, kc)()
            te.wait_ge(dmas, 80)  # whh0
            sel0(1)()
            for kc in range(2):
                g0(1, kc)()

            for t in range(S + D):
                # l0 scan step t
                if 0 < t < S:
                    te.wait_ge(a0, t)
                    for jc in range(4):
                        for kc in range(4):
                            mm = te.matmul(
                                ps0[(t // CH) % 2][:, jc, t % CH, :],
                                whh0[:, kc, jc, :],
                                hist0[:, kc, t - 1, :],
                                start=False, stop=(kc == 3),
                                skip_group_check=True,
                            )
                    mm.then_inc(h0)
                pop_ready(t, 2)
                # l1 scan step s
                s = t - D
                if 0 < s < S:
                    te.wait_ge(a1, s)
                    for jc in range(4):
                        for kc in range(4):
                            mm = te.matmul(
                                ps1[(s // CH) % 2][:, jc, s % CH, :],
                                whh1[:, kc, jc, :],
                                h1r[:, kc, (s - 1) % 2, :],
                                start=False, stop=(kc == 3),
                                skip_group_check=True,
                            )
                    mm.then_inc(h1)
                pop_ready(t, 2)

            # FC tail
            te.wait_ge(a1, S)
            te.wait_ge(dmas, 176)
            for kc in range(4):
                mm = te.matmul(
                    fcps[0:C, :], wfc[:, kc, :], h1r[:, kc, (S - 1) % 2, :],
                    start=(kc == 0), stop=(kc == 3),
                )
            mm.then_inc(fcs)

        @block.scalar
        def _(sc):
            for t in range(S + D):
                if t < S:
                    sc.wait_ge(h0, t + 1)
                    sc.activation(
                        hist0[:, :, t, :],
                        ps0[(t // CH) % 2][:, :, t % CH, :],
                        Tanh,
                    ).then_inc(a0)
                s = t - D
                if 0 <= s < S:
                    sc.wait_ge(h1, s + 1)
                    sc.activation(
                        h1r[:, :, s % 2, :],
                        ps1[(s // CH) % 2][:, :, s % CH, :],
                        Tanh,
                    ).then_inc(a1)

        @block.vector
        def _(ve):
            ve.wait_ge(fcs, 1)
            ve.tensor_scalar_add(fco[:, :], fcps[0:C, :], bfcv[:, 0:1]).then_inc(vs)

    nc.compile()
    return nc


def _make_sel():
    bf = ml_dtypes.bfloat16
    sel = np.zeros((4, 4, CH, BL), dtype=np.float32)
    for jc in range(4):
        sel[jc, jc, :, :] = 1.0
    return sel.astype(bf)


def _prep_inputs(inputs):
    bf = ml_dtypes.bfloat16
    w_ih0 = inputs["w_ih0"]
    w_hh0 = inputs["w_hh0"]
    w_ih1 = inputs["w_ih1"]
    w_hh1 = inputs["w_hh1"]
    w_fc = inputs["w_fc"]

    def lhsT_4(w, n_kc):
        # w: [512, n_kc*128] -> [kp, kc, jc, jp]
        return np.ascontiguousarray(
            w.reshape(4, 128, n_kc, 128).transpose(3, 2, 0, 1)
        ).astype(bf)

    shared = {
        "wih0": lhsT_4(w_ih0, 2),
        "whh0": lhsT_4(w_hh0, 4),
        "wih1": lhsT_4(w_ih1, 4),
        "whh1": lhsT_4(w_hh1, 4),
        "wfc": np.ascontiguousarray(
            w_fc.reshape(C, 4, 128).transpose(2, 1, 0)
        ).astype(bf),
        "b0": np.ascontiguousarray(
            (inputs["b_ih0"] + inputs["b_hh0"]).reshape(4, 128)
        ).astype(bf),
        "b1": np.ascontiguousarray(
            (inputs["b_ih1"] + inputs["b_hh1"]).reshape(4, 128)
        ).astype(bf),
        "sel": _make_sel(),
        "bfc": inputs["b_fc"].reshape(C, 1).astype(np.float32),
    }
    x = inputs["x"]
    in_maps = []
    for c in range(NCORES):
        xs = x[c * BL : (c + 1) * BL]  # [b, t, i]
        xT = (
            np.ascontiguousarray(
                xs.transpose(2, 1, 0).reshape(2, 128, S * BL).transpose(1, 0, 2)
            )
        ).astype(bf)
        m = dict(shared)
        m["xT"] = xT
        in_maps.append(m)
    return in_maps


def kernel(**inputs):
    from concourse import bass_utils

    if "nc" not in _cache:
        _cache["nc"] = _build_nc()
    nc = _cache["nc"]
    in_maps = _prep_inputs(inputs)
    res = bass_utils.run_bass_kernel_spmd(nc, in_maps, core_ids=list(range(NCORES)))
    y = np.concatenate(
        [np.asarray(res.results[c]["out"]).T for c in range(NCORES)], axis=0
    )
    return y.astype(np.float32)


# revision 18
# speedup vs baseline: 1.0972x; 1.0136x over previous
import numpy as np
import ml_dtypes

B, S, I, H, C = 64, 512, 256, 512, 10
NCORES = 8
BL = B // NCORES
D = 16          # l1 lag behind l0, in steps
CH = 8          # steps per pre-GEMM chunk (= one PSUM bank)
NCHK = S // CH

_cache = {}


def _build_nc():
    from collections import deque

    import concourse.bass as bass
    import concourse.bacc as bacc
    from concourse.bass import mybir

    f32 = mybir.dt.float32
    bf16 = mybir.dt.bfloat16
    Tanh = mybir.ActivationFunctionType.Tanh

    nc = bacc.Bacc("TRN2", target_bir_lowering=False, debug=False, num_devices=NCORES)

    xT_d = nc.dram_tensor("xT", [128, 2, S * BL], bf16, kind="ExternalInput")
    wih0_d = nc.dram_tensor("wih0", [128, 2, 4, 128], bf16, kind="ExternalInput")
    whh0_d = nc.dram_tensor("whh0", [128, 4, 4, 128], bf16, kind="ExternalInput")
    wih1_d = nc.dram_tensor("wih1", [128, 4, 4, 128], bf16, kind="ExternalInput")
    whh1_d = nc.dram_tensor("whh1", [128, 4, 4, 128], bf16, kind="ExternalInput")
    wfc_d = nc.dram_tensor("wfc", [128, 4, C], bf16, kind="ExternalInput")
    b0_d = nc.dram_tensor("b0", [4, 128], bf16, kind="ExternalInput")
    b1_d = nc.dram_tensor("b1", [4, 128], bf16, kind="ExternalInput")
    ones_d = nc.dram_tensor("sel", [4, 4, CH, BL], bf16, kind="ExternalInput")
    bfc_d = nc.dram_tensor("bfc", [C, 1], f32, kind="ExternalInput")
    out_d = nc.dram_tensor("out", [C, BL], f32, kind="ExternalOutput")
    import os
    DBG = os.environ.get("KDBG") == "1"
    if DBG:
        hist_d = nc.dram_tensor("hist", [128, 4, S, BL], bf16, kind="ExternalOutput")


    from contextlib import ExitStack

    with ExitStack() as stack:
        e = stack.enter_context
        block = e(nc.Block())
        dmas = e(nc.semaphore("dmas"))
        h0 = e(nc.semaphore("h0"))
        h1 = e(nc.semaphore("h1"))
        h0y = e(nc.semaphore("h0y"))
        h1y = e(nc.semaphore("h1y"))
        a0 = e(nc.semaphore("a0"))
        a1 = e(nc.semaphore("a1"))
        a0y = e(nc.semaphore("a0y"))
        a1y = e(nc.semaphore("a1y"))
        fcs = e(nc.semaphore("fcs"))
        vs = e(nc.semaphore("vs"))
        xT = e(nc.sbuf_tensor("xT_s", [128, 2, S * BL], bf16))
        wih0 = e(nc.sbuf_tensor("wih0_s", [128, 2, 4, 128], bf16))
        whh0 = e(nc.sbuf_tensor("whh0_s", [128, 4, 4, 128], bf16))
        wih1 = e(nc.sbuf_tensor("wih1_s", [128, 4, 4, 128], bf16))
        whh1 = e(nc.sbuf_tensor("whh1_s", [128, 4, 4, 128], bf16))
        wfc = e(nc.sbuf_tensor("wfc_s", [128, 4, C], bf16))
        b0r = e(nc.sbuf_tensor("b0r", [4, 128], bf16))
        b1r = e(nc.sbuf_tensor("b1r", [4, 128], bf16))
        onesv = e(nc.sbuf_tensor("selv", [4, 4, CH, BL], bf16))
        bfcv = e(nc.sbuf_tensor("bfcv_s", [C, 1], f32))
        hist0 = e(nc.sbuf_tensor("hist0", [128, 4, S, BL], bf16))
        h1r = e(nc.sbuf_tensor("h1r", [128, 4, 2, BL], bf16))
        fco = e(nc.sbuf_tensor("fco", [C, BL], f32))
        ps0a = e(nc.psum_tensor("ps0a", [128, 4, CH, BL], f32))
        ps0b = e(nc.psum_tensor("ps0b", [128, 4, CH, BL], f32))
        ps1a = e(nc.psum_tensor("ps1a", [128, 4, CH, BL], f32))
        ps1b = e(nc.psum_tensor("ps1b", [128, 4, CH, BL], f32))
        ps0 = [ps0a, ps0b]
        ps1 = [ps1a, ps1b]
        fcps = e(nc.psum_tensor("fcps", [128, BL], f32))
        # DMA milestones (each dma_start incs dmas by 16)
        # M1 (=64):  wih0, b0r, onesv, xT[0:64 steps]
        # M2 (=80):  whh0
        # M3 (=128): wih1, b1r, whh1
        # M4 (=144): xT rest
        # M5 (=176): wfc, bfcv
        @block.sync
        def _(sync):
            sync.dma_start(out=wih0[:], in_=wih0_d[:]).then_inc(dmas, 16)
            sync.dma_start(out=b0r[:], in_=b0_d[:]).then_inc(dmas, 16)
            sync.dma_start(out=onesv[:], in_=ones_d[:]).then_inc(dmas, 16)
            sync.dma_start(
                out=xT[:, :, 0 : 64 * BL], in_=xT_d[:, :, 0 : 64 * BL]
            ).then_inc(dmas, 16)
            sync.dma_start(out=whh0[:], in_=whh0_d[:]).then_inc(dmas, 16)
            sync.dma_start(out=wih1[:], in_=wih1_d[:]).then_inc(dmas, 16)
            sync.dma_start(out=b1r[:], in_=b1_d[:]).then_inc(dmas, 16)
            sync.dma_start(out=whh1[:], in_=whh1_d[:]).then_inc(dmas, 16)
            sync.dma_start(
                out=xT[:, :, 64 * BL :], in_=xT_d[:, :, 64 * BL :]
            ).then_inc(dmas, 16)
            sync.dma_start(out=wfc[:], in_=wfc_d[:]).then_inc(dmas, 16)
            sync.dma_start(out=bfcv[:], in_=bfc_d[:]).then_inc(dmas, 16)
            sync.wait_ge(vs, 1)
            sync.dma_start(out=out_d[:], in_=fco[:]).then_inc(dmas, 16)
            if DBG:
                sync.dma_start(out=hist_d[:], in_=hist0[:]).then_inc(dmas, 16)
                sync.wait_ge(dmas, 208)
            else:
                sync.wait_ge(dmas, 192)

        @block.tensor
        def _(te):
            te.wait_ge(dmas, 64)

            def sel0(c):
                def f():
                    if c == 8:
                        te.wait_ge(dmas, 144)  # xT rest
                    if c >= 2:
                        te.wait_ge(a0y, max(0, CH * (c - 1)))
                    te.matmul(
                        ps0[c % 2][:, :, :, :], b0r[:, :], onesv[:, :, :, :],
                        start=True, stop=False, skip_group_check=True,
                    )
                return f

            def g0(c, kc):
                def f():
                    for jc in range(4):
                        mm = te.matmul(
                            ps0[c % 2][:, jc, :, :],
                            wih0[:, kc, jc, :],
                            xT[:, kc, c * CH * BL : (c + 1) * CH * BL],
                            start=False, stop=False, skip_group_check=True,
                        )
                        if c == 0 and kc == 1 and jc == 2:
                            mm.then_inc(h0)
                        if c == 0 and kc == 1 and jc == 3:
                            mm.then_inc(h0y)
                return f

            def sel1(c):
                def f():
                    if c == 0:
                        te.wait_ge(dmas, 128)  # wih1, b1r, whh1
                    if c >= 2:
                        te.wait_ge(a1y, max(0, CH * (c - 1)))
                    te.matmul(
                        ps1[c % 2][:, :, :, :], b1r[:, :], onesv[:, :, :, :],
                        start=True, stop=False, skip_group_check=True,
                    )
                return f

            def g1(c, kc):
                def f():
                    if kc == 0:
                        te.wait_ge(a0y, CH * c + CH)  # h0 history through chunk
                    for jc in range(4):
                        mm = te.matmul(
                            ps1[c % 2][:, jc, :, :],
                            wih1[:, kc, jc, :],
                            hist0[:, kc, c * CH : (c + 1) * CH, :],
                            start=False, stop=False, skip_group_check=True,
                        )
                        if c == 0 and kc == 3 and jc == 2:
                            mm.then_inc(h1)
                        if c == 0 and kc == 3 and jc == 3:
                            mm.then_inc(h1y)
                return f

            q = deque()
            for c in range(2, NCHK):
                q.append((CH * c - 8, sel0(c)))
                for kc in range(2):
                    q.append((CH * c - 6 + kc, g0(c, kc)))
            q1 = deque()
            for c in range(NCHK):
                q1.append((CH * c + D - 8, sel1(c)))
                for kc in range(4):
                    q1.append((CH * c + D - 6 + kc, g1(c, kc)))

            def pop_ready(t, n):
                done = 0
                while done < n:
                    if q1 and q1[0][0] <= t:
                        q1.popleft()[1]()
                    elif q and q[0][0] <= t:
                        q.popleft()[1]()
                    else:
                        break
                    done += 1

            # prime: l0 chunks 0 and 1
            sel0(0)()
            for kc in range(2):
                g0(0, kc)()
            te.wait_ge(dmas, 80)  # whh0
            sel0(1)()
            for kc in range(2):
                g0(1, kc)()

            for t in range(S + D):
                # l0 scan step t
                if 0 < t < S:
                    te.wait_ge(a0, t)
                    for kc in range(2):
                        for jc in range(4):
                            te.matmul(
                                ps0[(t // CH) % 2][:, jc, t % CH, :],
                                whh0[:, kc, jc, :],
                                hist0[:, kc, t - 1, :],
                                start=False, stop=False,
                                skip_group_check=True,
                            )
                    te.wait_ge(a0y, t)
                    for kc in range(2, 4):
                        for jc in range(4):
                            mm = te.matmul(
                                ps0[(t // CH) % 2][:, jc, t % CH, :],
                                whh0[:, kc, jc, :],
                                hist0[:, kc, t - 1, :],
                                start=False, stop=(kc == 3),
                                skip_group_check=True,
                            )
                            if kc == 3 and jc == 1:
                                mm.then_inc(h0)
                    mm.then_inc(h0y)
                pop_ready(t, 2)
                # l1 scan step s
                s = t - D
                if 0 < s < S:
                    te.wait_ge(a1, s)
                    for kc in range(2):
                        for jc in range(4):
                            te.matmul(
                                ps1[(s // CH) % 2][:, jc, s % CH, :],
                                whh1[:, kc, jc, :],
                                h1r[:, kc, (s - 1) % 2, :],
                                start=False, stop=False,
                                skip_group_check=True,
                            )
                    te.wait_ge(a1y, s)
                    for kc in range(2, 4):
                        for jc in range(4):
                            mm = te.matmul(
                                ps1[(s // CH) % 2][:, jc, s % CH, :],
                                whh1[:, kc, jc, :],
                                h1r[:, kc, (s - 1) % 2, :],
                                start=False, stop=(kc == 3),
                                skip_group_check=True,
                            )
                            if kc == 3 and jc == 1:
                                mm.then_inc(h1)
                    mm.then_inc(h1y)
                pop_ready(t, 2)

            # FC tail
            te.wait_ge(a1y, S)
            te.wait_ge(dmas, 176)
            for kc in range(4):
                mm = te.matmul(
                    fcps[0:C, :], wfc[:, kc, :], h1r[:, kc, (S - 1) % 2, :],
                    start=(kc == 0), stop=(kc == 3),
                )
            mm.then_inc(fcs)

        @block.scalar
        def _(sc):
            for t in range(S + D):
                if t < S:
                    sc.wait_ge(h0, t + 1)
                    sc.activation(
                        hist0[:, 0:2, t, :],
                        ps0[(t // CH) % 2][:, 0:2, t % CH, :],
                        Tanh,
                    ).then_inc(a0)
                    sc.wait_ge(h0y, t + 1)
                    sc.activation(
                        hist0[:, 2:4, t, :],
                        ps0[(t // CH) % 2][:, 2:4, t % CH, :],
                        Tanh,
                    ).then_inc(a0y)
                s = t - D
                if 0 <= s < S:
                    sc.wait_ge(h1, s + 1)
                    sc.activation(
                        h1r[:, 0:2, s % 2, :],
                        ps1[(s // CH) % 2][:, 0:2, s % CH, :],
                        Tanh,
                    ).then_inc(a1)
                    sc.wait_ge(h1y, s + 1)
                    sc.activation(
                        h1r[:, 2:4, s % 2, :],
                        ps1[(s // CH) % 2][:, 2:4, s % CH, :],
                        Tanh,
                    ).then_inc(a1y)

        @block.vector
        def _(ve):
            ve.wait_ge(fcs, 1)
            ve.tensor_scalar_add(fco[:, :], fcps[0:C, :], bfcv[:, 0:1]).then_inc(vs)

    nc.compile()
    return nc


def _make_sel():
    bf = ml_dtypes.bfloat16
    sel = np.zeros((4, 4, CH, BL), dtype=np.float32)
    for jc in range(4):
        sel[jc, jc, :, :] = 1.0
    return sel.astype(bf)


def _prep_inputs(inputs):
    bf = ml_dtypes.bfloat16
    w_ih0 = inputs["w_ih0"]
    w_hh0 = inputs["w_hh0"]
    w_ih1 = inputs["w_ih1"]
    w_hh1 = inputs["w_hh1"]
    w_fc = inputs["w_fc"]

    def lhsT_4(w, n_kc):
        # w: [512, n_kc*128] -> [kp, kc, jc, jp]
        return np.ascontiguousarray(
            w.reshape(4, 128, n_kc, 128).transpose(3, 2, 0, 1)
        ).astype(bf)

    shared = {
        "wih0": lhsT_4(w_ih0, 2),
        "whh0": lhsT_4(w_hh0, 4),
        "wih1": lhsT_4(w_ih1, 4),
        "whh1": lhsT_4(w_hh1, 4),
        "wfc": np.ascontiguousarray(
            w_fc.reshape(C, 4, 128).transpose(2, 1, 0)
        ).astype(bf),
        "b0": np.ascontiguousarray(
            (inputs["b_ih0"] + inputs["b_hh0"]).reshape(4, 128)
        ).astype(bf),
        "b1": np.ascontiguousarray(
            (inputs["b_ih1"] + inputs["b_hh1"]).reshape(4, 128)
        ).astype(bf),
        "sel": _make_sel(),
        "bfc": inputs["b_fc"].reshape(C, 1).astype(np.float32),
    }
    x = inputs["x"]
    in_maps = []
    for c in range(NCORES):
        xs = x[c * BL : (c + 1) * BL]  # [b, t, i]
        xT = (
            np.ascontiguousarray(
                xs.transpose(2, 1, 0).reshape(2, 128, S * BL).transpose(1, 0, 2)
            )
        ).astype(bf)
        m = dict(shared)
        m["xT"] = xT
        in_maps.append(m)
    return in_maps


def kernel(**inputs):
    from concourse import bass_utils

    if "nc" not in _cache:
        _cache["nc"] = _build_nc()
    nc = _cache["nc"]
    in_maps = _prep_inputs(inputs)
    res = bass_utils.run_bass_kernel_spmd(nc, in_maps, core_ids=list(range(NCORES)))
    y = np.concatenate(
        [np.asarray(res.results[c]["out"]).T for c in range(NCORES)], axis=0
    )
    return y.astype(np.float32)
